# revision 13
# baseline (speedup 1.0000x reference)
"""Spatial attention block (GroupNorm + QKV 1x1 + full spatial attention +
out-proj + residual) on 8 Trainium2 NeuronCores — v2.

Sharding: core = (batch b, spatial quarter j); each core's input is rotated
along the flattened spatial axis so the SPMD program computes the first 1024
query positions against all 4096 keys (attention is invariant to a joint
rotation of the key/value axis; GroupNorm stats are rotation-invariant).

Device program highlights:
 - GroupNorm is applied as a per-channel scale+bias fused into a Pool-engine
   fp8 cast of x (x8), freeing ACT/DVE for softmax work.
 - K/Q/V projections run as fp8 DoubleRow matmuls (256-channel contraction in
   one instruction) reading x8; K/Q evacuate to bf16, V to an fp8 [V|1]
   layout whose ones column accumulates softmax denominators during AV.
 - QK^T runs in bf16 (64-row contraction); scores exp is split per chunk:
   ACT computes exact exp -> fp8, DVE computes a Schraudolph-style exp that
   writes fp8 bytes directly through a saturating uint8 store.
 - AV runs as fp8 DoubleRow (two 128-key chunks per instruction).
 - Per-head normalize: denominator row -> f32r, ones-broadcast matmul,
   reciprocal_approx_fast, one DVE multiply.
"""

import sys

for _p in ("/opt/trn_rl_repo", "/root/.axon_site/_ro/trn_rl_repo"):
    if _p not in sys.path:
        sys.path.insert(0, _p)

import numpy as np

import concourse.bacc as bacc
import concourse.bass as bass
import concourse.tile as tile
from concourse import mybir
from concourse.bass_utils import run_bass_kernel_spmd

F32 = mybir.dt.float32
F32R = mybir.dt.float32r
BF16 = mybir.dt.bfloat16
FP8 = mybir.dt.float8e4
U8 = mybir.dt.uint8
AF = mybir.ActivationFunctionType
DR = mybir.MatmulPerfMode.DoubleRow

B, C, H, W = 2, 256, 64, 64
S = H * W              # 4096 spatial positions
NH = 4                 # heads
HD = C // NH           # 64 head dim
NQ = S // 4            # 1024 query positions per core
NDC = S // 256         # 16 double-chunks of 256 keys
EPS = 1e-5
WSCALE = 4.0           # host scales W_{q,k,v} by this before fp8 quantization
# exp(s_true/16); psum scores = (4q)(4k) = 16 s_true
SCALE_EXP = 1.0 / (16.0 * WSCALE * WSCALE)
A_DVE = 8 * np.log2(np.e) * SCALE_EXP   # schraudolph: byte = s*A + B
B_DVE = 56.0 - 0.37
# dc indices where ACT also handles the second chunk (load balancing)
ACT_STEAL = (5, 11)


def _build_program():
    nc = bacc.Bacc(None)

    x_d = nc.declare_dram_parameter("x", [C, S], F32, isOutput=False)
    wk8_d = nc.declare_dram_parameter("wk8", [128, 2 * 2 * 128], FP8, isOutput=False)
    wq8_d = nc.declare_dram_parameter("wq8", [128, 2 * 2 * 128], FP8, isOutput=False)
    wv8_d = nc.declare_dram_parameter("wv8", [128, 2 * 256], FP8, isOutput=False)
    woT_d = nc.declare_dram_parameter("woT", [NH, HD, C], BF16, isOutput=False)
    gnw_d = nc.declare_dram_parameter("gnw", [2, 128, 1], F32, isOutput=False)
    gnb_d = nc.declare_dram_parameter("gnb", [2, 128, 1], F32, isOutput=False)
    ob_d = nc.declare_dram_parameter("ob", [2, 128, 1], F32, isOutput=False)
    gsel_d = nc.declare_dram_parameter("gsel", [128, 8], F32R, isOutput=False)
    gselT_d = nc.declare_dram_parameter("gselT", [8, 128], F32R, isOutput=False)
    y_d = nc.declare_dram_parameter("y", [C, NQ], F32, isOutput=True)

    with tile.TileContext(nc) as tc, nc.allow_low_precision("fp8/f32r kernel"):
        _emit(nc, tc, x_d, wk8_d, wq8_d, wv8_d, woT_d, gnw_d, gnb_d, ob_d,
              gsel_d, gselT_d, y_d)
    nc.finalize()
    return nc


def _emit(nc, tc, x_d, wk8_d, wq8_d, wv8_d, woT_d, gnw_d, gnb_d, ob_d,
          gsel_d, gselT_d, y_d):
    from contextlib import ExitStack

    ctx = ExitStack()
    with ctx:
        persist = ctx.enter_context(tc.tile_pool(name="persist", bufs=1))
        qk = ctx.enter_context(tc.tile_pool(name="qk", bufs=4, space="PSUM"))
        av = ctx.enter_context(tc.tile_pool(name="av", bufs=1, space="PSUM"))

        # ---- persistent SBUF tiles -------------------------------------
        x_sb = [persist.tile([128, S], BF16, tag=f"x{t}", name=f"x{t}") for t in range(2)]
        x8 = persist.tile([128, 2, S], FP8, tag="x8")
        k_sb = [persist.tile([128, S], BF16, tag=f"k{t}", name=f"k{t}") for t in range(2)]
        q_sb = [persist.tile([128, NQ], BF16, tag=f"q{t}", name=f"q{t}") for t in range(2)]
        vt8 = [persist.tile([128, 2, NH, 80], FP8, tag=f"vt{c}", name=f"vt{c}") for c in range(NDC)]
        attn_sb = [persist.tile([64, NQ], BF16, tag=f"at{h}", name=f"at{h}") for h in range(NH)]
        wk8_sb = persist.tile([128, 2, 2 * 128], FP8, tag="wk8")
        wq8_sb = persist.tile([128, 2, 2 * 128], FP8, tag="wq8")
        wv8_sb = persist.tile([128, 2, 256], FP8, tag="wv8")
        wo_sb = [persist.tile([HD, C], BF16, tag=f"wo{h}", name=f"wo{h}") for h in range(NH)]
        gnw_sb = [persist.tile([128, 1], F32, tag=f"gw{t}", name=f"gw{t}") for t in range(2)]
        gnb_sb = [persist.tile([128, 1], F32, tag=f"gb{t}", name=f"gb{t}") for t in range(2)]
        ob_sb = [persist.tile([128, 1], F32, tag=f"ob{t}", name=f"obb{t}") for t in range(2)]
        gsel_sb = persist.tile([128, 8], F32R, tag="gsel")
        gselT_sb = persist.tile([8, 128], F32R, tag="gselT")
        oacc_sb = [persist.tile([128, NQ], F32, tag=f"oacc{t}", name=f"oacc{t}") for t in range(2)]
        out_sb = [persist.tile([128, NQ], F32, tag=f"out{t}", name=f"outt{t}") for t in range(2)]
        eps_sb = persist.tile([128, 1], F32, tag="eps")
        warm_sb = persist.tile([128, 512], BF16, tag="warm")
        ones64 = persist.tile([65, 64], F32R, tag="ones64")
        dnr = [persist.tile([65, NQ], F32R, tag=f"dnr{i}", name=f"dnr{i}") for i in range(2)]
        rinv = [persist.tile([64, NQ], F32, tag=f"rinv{i}", name=f"rinv{i}") for i in range(2)]

        nc.vector.memset(eps_sb, EPS)
        nc.vector.memset(warm_sb, 0.0)

        # ---- input DMAs ------------------------------------------------
        for xc in range(4):
            for t in range(2):
                nc.gpsimd.dma_start(
                    out=x_sb[t][:, 1024 * xc : 1024 * (xc + 1)],
                    in_=x_d[128 * t : 128 * (t + 1), 1024 * xc : 1024 * (xc + 1)],
                )
        for t in range(2):
            nc.sync.dma_start(out=gnw_sb[t], in_=gnw_d[t])
            nc.sync.dma_start(out=gnb_sb[t], in_=gnb_d[t])
            nc.sync.dma_start(out=ob_sb[t], in_=ob_d[t])
        nc.sync.dma_start(out=wk8_sb, in_=wk8_d[:].rearrange("p (s n) -> p s n", s=2))
        nc.sync.dma_start(out=wq8_sb, in_=wq8_d[:].rearrange("p (s n) -> p s n", s=2))
        nc.sync.dma_start(out=wv8_sb, in_=wv8_d[:].rearrange("p (s n) -> p s n", s=2))
        for h in range(NH):
            nc.sync.dma_start(out=wo_sb[h], in_=woT_d[h])
        nc.sync.dma_start(out=gsel_sb, in_=gsel_d[:])
        nc.sync.dma_start(out=gselT_sb, in_=gselT_d[:])

        # ones column of [V|1] tiles; ones row (partition 64) for bcasts
        for c in range(NDC):
            nc.vector.memset(vt8[c][:, :, :, 64:65], 1.0)

        warm_n = [0]

        def emit_warm(n):
            d = qk.tile([128, 256], F32, tag="ps", name=f"warm{warm_n[0]}")
            warm_n[0] += 1
            for _ in range(n):
                nc.tensor.matmul(d, warm_sb[:, 0:128], warm_sb[:, 0:256],
                                 start=True, stop=True)

        emit_warm(30)
        # data-dependent warms: fire as each x quarter lands, spreading PE
        # activity across the DMA phase so the HAM gate stays released
        for xc in range(4):
            for t in range(2):
                wd = qk.tile([128, 256], F32, tag="ps", name=f"wdep{xc}_{t}")
                for _ in range(5):
                    nc.tensor.matmul(wd, x_sb[t][:, 1024 * xc : 1024 * xc + 128],
                                     x_sb[t][:, 1024 * xc : 1024 * xc + 256],
                                     start=True, stop=True)

        # ---- GroupNorm stats -> per-channel scale/bias -----------------
        with tc.tile_pool(name="gnp", bufs=1) as gnp:
            s_t, b_t = [], []
            for t in range(2):
                nsub = S // 512
                st6 = gnp.tile([128, nsub, 6], F32, tag=f"st6_{t}")
                for i in range(nsub):
                    nc.vector.bn_stats(out=st6[:, i, :], in_=x_sb[t][:, 512 * i : 512 * (i + 1)])
                mv = gnp.tile([128, 2], F32, tag=f"mv{t}")
                nc.vector.bn_aggr(out=mv, in_=st6)
                stats2 = gnp.tile([128, 2], F32R, tag=f"s2_{t}")
                nc.vector.tensor_copy(out=stats2[:, 0:1], in_=mv[:, 0:1])
                nc.vector.tensor_tensor(out=stats2[:, 1:2], in0=mv[:, 0:1], in1=mv[:, 0:1],
                                        op=mybir.AluOpType.mult)
                nc.vector.tensor_tensor(out=stats2[:, 1:2], in0=stats2[:, 1:2], in1=mv[:, 1:2],
                                        op=mybir.AluOpType.add)
                pg = qk.tile([8, 2], F32, tag="ps", name=f"pg{t}")
                nc.tensor.matmul(pg, gsel_sb, stats2, start=True, stop=True)
                g2 = gnp.tile([8, 2], F32, tag=f"g2_{t}")
                nc.scalar.activation(out=g2, in_=pg, func=AF.Copy, scale=1.0 / 16.0)
                mr = gnp.tile([8, 2], F32R, tag=f"mr{t}")
                nc.vector.tensor_copy(out=mr[:, 0:1], in_=g2[:, 0:1])
                vg = gnp.tile([8, 1], F32, tag=f"vg{t}")
                nc.vector.tensor_tensor(out=vg, in0=g2[:, 0:1], in1=g2[:, 0:1],
                                        op=mybir.AluOpType.mult)
                nc.vector.tensor_tensor(out=vg, in0=g2[:, 1:2], in1=vg,
                                        op=mybir.AluOpType.subtract)
                nc.vector.tensor_scalar(out=vg, in0=vg, scalar1=EPS, scalar2=None,
                                        op0=mybir.AluOpType.add)
                vg2 = gnp.tile([8, 1], F32, tag=f"vg2_{t}")
                nc.vector.reciprocal_approx_fast(out=vg2, in_=vg)
                nc.scalar.activation(out=mr[:, 1:2], in_=vg2, func=AF.Sqrt)
                pb = qk.tile([128, 2], F32, tag="ps", name=f"pb{t}")
                nc.tensor.matmul(pb, gselT_sb, mr, start=True, stop=True)
                sc = gnp.tile([128, 1], F32, tag=f"sc{t}")
                bi = gnp.tile([128, 1], F32, tag=f"bi{t}")
                nc.vector.tensor_tensor(out=sc, in0=gnw_sb[t], in1=pb[:, 1:2],
                                        op=mybir.AluOpType.mult)
                nc.vector.tensor_tensor(out=bi, in0=pb[:, 0:1], in1=sc,
                                        op=mybir.AluOpType.mult)
                nc.vector.tensor_tensor(out=bi, in0=gnb_sb[t], in1=bi,
                                        op=mybir.AluOpType.subtract)
                s_t.append(sc)
                b_t.append(bi)
                emit_warm(10)

            # ones row at partition 64 (f32r) for denominator broadcasts
            nc.scalar.activation(out=ones64[64:65, :], in_=x_sb[0][64:65, 0:64],
                                 func=AF.Identity, scale=0.0, bias=1.0)

            # residual + out_bias accumulator
            for t in range(2):
                nc.vector.tensor_scalar(out=oacc_sb[t], in0=x_sb[t][:, 0:NQ],
                                        scalar1=ob_sb[t], scalar2=None,
                                        op0=mybir.AluOpType.add)

            # ---- GN-applied fp8 cast of x (Pool engine) ----------------
            for t in range(2):
                eng = nc.gpsimd if t == 0 else nc.vector
                for b in range(S // 512):
                    eng.tensor_scalar(
                        out=x8[:, t, 512 * b : 512 * (b + 1)],
                        in0=x_sb[t][:, 512 * b : 512 * (b + 1)],
                        scalar1=s_t[t], scalar2=b_t[t],
                        op0=mybir.AluOpType.mult, op1=mybir.AluOpType.add,
                    )
                    if b % 2 == 0:
                        emit_warm(2)

            # ---- projections (fp8 DoubleRow) ---------------------------
            evac_n = [0]

            def evac(dst, src, scale=None):
                # alternate ACT/DVE for PSUM->SBUF evacuations
                if evac_n[0] % 2 == 0:
                    nc.scalar.activation(out=dst, in_=src, func=AF.Copy,
                                         scale=1.0 if scale is None else scale)
                else:
                    if scale is None:
                        nc.vector.tensor_copy(out=dst, in_=src)
                    else:
                        nc.vector.tensor_scalar(out=dst, in0=src, scalar1=scale,
                                                scalar2=None, op0=mybir.AluOpType.mult)
                evac_n[0] += 1

            # K/Q for pair tile 0 first so attention can start early
            for t in range(2):
                for b in range(S // 512):
                    pk = qk.tile([128, 512], F32, tag="ps", name=f"pk{t}_{b}")
                    nc.tensor.matmul(pk, wk8_sb[:, :, 128 * t : 128 * (t + 1)],
                                     x8[:, :, 512 * b : 512 * (b + 1)],
                                     start=True, stop=True, perf_mode=DR)
                    evac(k_sb[t][:, 512 * b : 512 * (b + 1)], pk)
                    emit_warm(1)
                for nb in range(NQ // 512):
                    pq = qk.tile([128, 512], F32, tag="ps", name=f"pq{t}_{nb}")
                    nc.tensor.matmul(pq, wq8_sb[:, :, 128 * t : 128 * (t + 1)],
                                     x8[:, :, 512 * nb : 512 * (nb + 1)],
                                     start=True, stop=True, perf_mode=DR)
                    evac(q_sb[t][:, 512 * nb : 512 * (nb + 1)], pq)
            # V
            for c in range(2 * NDC):
                pv = qk.tile([128, 256], F32, tag="ps", name=f"pv{c}")
                nc.tensor.matmul(pv, x8[:, :, 128 * c : 128 * (c + 1)], wv8_sb,
                                 start=True, stop=True, perf_mode=DR)
                evac(vt8[c // 2][:, c % 2, :, 0:64],
                     pv.rearrange("p (h d) -> p h d", h=NH), scale=1.0 / WSCALE)
                if c % 2 == 0:
                    emit_warm(1)

        # ---- attention -------------------------------------------------
        # Head pairs (k/q tile = pair). Scores psum is [128, 512]-grained,
        # 8 slots = two key-chunks in flight. ACT computes exact exp->fp8 for
        # the even head, DVE a Schraudolph exp (saturating uint8 store of fp8
        # bytes) for the odd head. AV (fp8 DoubleRow) trails one double-chunk.
        with tc.tile_pool(name="ep", bufs=3) as ep:
            e8_live = {}
            po_pair = {}

            def emit_qk_exp(pair, dc):
                t = pair
                e8E, e8O = e8_live[(pair, dc)]
                for cc in range(2):
                    c = 2 * dc + cc
                    for i, e8t in ((0, e8E), (1, e8O)):
                        r = 64 * i
                        for nb in range(NQ // 512):
                            ps = qk.tile([128, 512], F32, tag="ps",
                                         name=f"ps{pair}_{c}_{i}_{nb}")
                            nc.tensor.matmul(
                                ps, k_sb[t][r : r + 64, 128 * c : 128 * (c + 1)],
                                q_sb[t][r : r + 64, 512 * nb : 512 * (nb + 1)],
                                start=True, stop=True,
                            )
                            if i == 0:
                                nc.scalar.activation(
                                    out=e8t[:, cc, 512 * nb : 512 * (nb + 1)],
                                    in_=ps, func=AF.Exp, scale=SCALE_EXP)
                            else:
                                nc.vector.tensor_scalar(
                                    out=e8t.bitcast(U8)[:, cc, 512 * nb : 512 * (nb + 1)],
                                    in0=ps, scalar1=float(A_DVE), scalar2=float(B_DVE),
                                    op0=mybir.AluOpType.mult, op1=mybir.AluOpType.add)

            def emit_av(pair, dc):
                e8E, e8O = e8_live.pop((pair, dc))
                poE, poO = po_pair[pair]
                for e8t, po, h in ((e8E, poE, 2 * pair), (e8O, poO, 2 * pair + 1)):
                    for nb in range(NQ // 512):
                        nc.tensor.matmul(
                            po[:, 512 * nb : 512 * (nb + 1)],
                            vt8[dc][:, :, h, 0:65],
                            e8t[:, :, 512 * nb : 512 * (nb + 1)],
                            start=(dc == 0), stop=(dc == NDC - 1), perf_mode=DR,
                        )

            def emit_norm_front(pair):
                poE, poO = po_pair[pair]
                for i, po in enumerate((poE, poO)):
                    nc.scalar.activation(out=dnr[i][64:65, :], in_=po[64:65, :],
                                         func=AF.Copy)

            def emit_norm_back(pair, warm=False):
                poE, poO = po_pair[pair]
                for i, po in enumerate((poE, poO)):
                    h = 2 * pair + i
                    if warm:
                        wd = qk.tile([128, 256], F32, tag="ps", name=f"wnb{h}")
                        for _ in range(6):
                            nc.tensor.matmul(wd, warm_sb[:, 0:128], warm_sb[:, 0:256],
                                             start=True, stop=True)
                    for nb in range(NQ // 512):
                        nbs = slice(512 * nb, 512 * (nb + 1))
                        pbc = qk.tile([128, 512], F32, tag="ps", name=f"pbc{h}_{nb}")
                        nc.tensor.matmul(pbc[0:64, :], ones64[64:65, :],
                                         dnr[i][64:65, nbs], start=True, stop=True)
                        nc.vector.reciprocal_approx_fast(out=rinv[i][:, nbs],
                                                         in_=pbc[0:64, :])
                    nc.vector.tensor_tensor(out=attn_sb[h], in0=po[0:64, :], in1=rinv[i],
                                            op=mybir.AluOpType.mult)

            for pair in range(2):
                po_pair[pair] = (
                    av.tile([65, NQ], F32, tag="poE", name=f"poE{pair}", bufs=1),
                    av.tile([65, NQ], F32, tag="poO", name=f"poO{pair}", bufs=1),
                )
                for dc in range(NDC):
                    e8_live[(pair, dc)] = (
                        ep.tile([128, 2, NQ], FP8, tag="e8E", name=f"e8E_{pair}_{dc}"),
                        ep.tile([128, 2, NQ], FP8, tag="e8O", name=f"e8O_{pair}_{dc}"),
                    )
                    emit_qk_exp(pair, dc)
                    if dc >= 2:
                        emit_av(pair, dc - 2)
                    if dc == 1 and pair == 1:
                        emit_norm_back(0)
                emit_av(pair, NDC - 2)
                emit_av(pair, NDC - 1)
                emit_norm_front(pair)
            emit_norm_back(1)

        # ---- out-proj + residual + store -------------------------------
        if True:
            # out-proj accumulators live in the (now idle) score slots so they
            # do not wait on the pair-1 AV accumulators being released; heads
            # 0/1 out-proj overlaps the pair-1 normalize chain.
            pf = {}
            for o in range(2):
                for nb in range(NQ // 512):
                    pf[(o, nb)] = qk.tile([128, 512], F32, tag="ps",
                                          name=f"pf{o}_{nb}")
            for h in range(NH):
                for o in range(2):
                    for nb in range(NQ // 512):
                        nc.tensor.matmul(
                            pf[(o, nb)],
                            wo_sb[h][:, 128 * o : 128 * (o + 1)],
                            attn_sb[h][:, 512 * nb : 512 * (nb + 1)],
                            start=(h == 0), stop=(h == NH - 1),
                        )
            for o in range(2):
                for nb in range(NQ // 512):
                    nbs = slice(512 * nb, 512 * (nb + 1))
                    nc.vector.tensor_tensor(out=out_sb[o][:, nbs], in0=oacc_sb[o][:, nbs],
                                            in1=pf[(o, nb)], op=mybir.AluOpType.add)
                    nc.sync.dma_start(out=y_d[128 * o : 128 * (o + 1), nbs],
                                      in_=out_sb[o][:, nbs])


_PROGRAM = None


def _get_program():
    global _PROGRAM
    if _PROGRAM is None:
        _PROGRAM = _build_program()
    return _PROGRAM


def _prep_inputs(input, gn_weight, gn_bias, qkv_weight, out_weight, out_bias):
    import ml_dtypes

    input = np.asarray(input, dtype=np.float32).reshape(B, C, S)
    gn_weight = np.asarray(gn_weight, dtype=np.float32)
    gn_bias = np.asarray(gn_bias, dtype=np.float32)
    qkv_weight = np.asarray(qkv_weight, dtype=np.float32)
    out_weight = np.asarray(out_weight, dtype=np.float32)
    out_bias = np.asarray(out_bias, dtype=np.float32)

    # reference packs qkv head-major: rows 192h..192h+192 = [q|k|v] of head h
    wq = np.stack([qkv_weight[192 * h + 0 : 192 * h + 64] for h in range(NH)])   # [NH, 64, 256]
    wk = np.stack([qkv_weight[192 * h + 64 : 192 * h + 128] for h in range(NH)])
    wv = np.stack([qkv_weight[192 * h + 128 : 192 * h + 192] for h in range(NH)])

    def dr_pack(w):  # [NH, 64, C_in] -> [128, 2, 256] (p, s) = ch 128s+p; cols head-major
        m = w.reshape(NH * HD, C).T  # [C_in, 256 out]
        m = m.reshape(2, 128, NH * HD).transpose(1, 0, 2)  # [p, s, out]
        return np.ascontiguousarray(m)

    clip = lambda a: np.clip(a, -240.0, 240.0)
    wk8 = clip(dr_pack(wk) * WSCALE).astype(ml_dtypes.float8_e4m3).reshape(128, -1)
    wq8 = clip(dr_pack(wq) * WSCALE).astype(ml_dtypes.float8_e4m3).reshape(128, -1)
    wv8 = clip(dr_pack(wv) * WSCALE).astype(ml_dtypes.float8_e4m3).reshape(128, -1)

    woT = np.ascontiguousarray(out_weight.T.reshape(NH, HD, C)).astype(ml_dtypes.bfloat16)
    gnw = np.ascontiguousarray(gn_weight.reshape(2, 128, 1))
    gnb = np.ascontiguousarray(gn_bias.reshape(2, 128, 1))
    ob = np.ascontiguousarray(out_bias.reshape(2, 128, 1))
    gsel = np.zeros((128, 8), np.float32)
    for p in range(128):
        gsel[p, p // 16] = 1.0
    gselT = np.ascontiguousarray(gsel.T)

    in_maps = []
    for core in range(8):
        b, j = core // 4, core % 4
        xrot = np.roll(input[b], -NQ * j, axis=1)
        in_maps.append({
            "x": np.ascontiguousarray(xrot),
            "wk8": wk8, "wq8": wq8, "wv8": wv8,
            "woT": woT, "gnw": gnw, "gnb": gnb, "ob": ob,
            "gsel": gsel, "gselT": gselT,
        })
    return in_maps


def kernel(input, gn_weight, gn_bias, qkv_weight, out_weight, out_bias, _trace=False):
    nc = _get_program()
    in_maps = _prep_inputs(input, gn_weight, gn_bias, qkv_weight, out_weight, out_bias)
    kw = {}
    if _trace:
        kw = {"trace": True, "tmpdir": "/tmp/attn_trace"}
    res = run_bass_kernel_spmd(nc, in_maps, list(range(8)), **kw)
    out = np.empty((B, C, S), np.float32)
    for core in range(8):
        b, j = core // 4, core % 4
        out[b, :, NQ * j : NQ * (j + 1)] = res.results[core]["y"]
    out = out.reshape(B, C, H, W)
    if _trace:
        return out, res
    return out
